# revision 25
# baseline (speedup 1.0000x reference)
"""Multi-head causal attention with RoPE on 8 TRN2 NeuronCores.

Sharding: batch (2) x head-groups (4 of 4 heads) -> 8 cores.
Per core, processed per 512-row s-chunk with everything interleaved to keep
the PE array dense (HAM stays at K=8/8): QKV projection for the chunk,
RoPE (stream_shuffle + sign-folded cos/sin), transposed scores
S^T = Kr @ Qr^T with both heads of a pair in one [128,1024] PSUM tile (the
heads share the Kr-block stationary; one fused scale+exp covers the pair),
causal block-skip plus column-subrange matmuls on the diagonal blocks, PV
matmul with a ones-column on V accumulating the softmax denominator,
ACT-side reciprocal (exp(-ln)), ones-matmul broadcast. O accumulators are
copied out of PSUM right after each head-pair (ACT) so two PSUM banks
suffice; the normalize chain runs deferred off the Tensor critical path,
and W_o runs a half-chunk behind. Host sums the 4 per-batch partials.
bf16 throughout; f32 PSUM accumulation.
"""
import os
import sys

sys.path.insert(0, "/opt/trn_rl_repo")

import ml_dtypes
import numpy as np

import concourse.bass as bass
import concourse.mybir as mybir
import concourse.tile as tile
from concourse import bass_utils

F32 = mybir.dt.float32
F32R = mybir.dt.float32r
BF16 = mybir.dt.bfloat16

DT_NAME = os.environ.get("ATTN_DT", "bf16")
DT = {"f32r": F32R, "bf16": BF16}[DT_NAME]
DT_NP = {"f32r": np.float32, "bf16": ml_dtypes.bfloat16}[DT_NAME]

B, S, E, H, Dh = 2, 2048, 1024, 16, 64
HG = 4            # heads per core
HD = HG * Dh      # 256 output channels per core
SCALE = float(1.0 / np.sqrt(np.float32(1024.0)))
ROPE_BASE = 10000.0
NCHUNK = S // 512     # 4 s-chunks of 512
NTB = S // 128        # 16 t-blocks of 128
SHUF16 = list(range(16, 32)) + list(range(0, 16))

Exp = mybir.ActivationFunctionType.Exp
Ln = mybir.ActivationFunctionType.Ln
MUL = mybir.AluOpType.mult
ADD = mybir.AluOpType.add


def _build_program():
    nc = bass.Bass("TRN2", target_bir_lowering=False, debug=False)

    xT = nc.dram_tensor("xT", [128, NCHUNK, 8, 512], DT, kind="ExternalInput")
    wq = nc.dram_tensor("wq", [128, 8, HD], DT, kind="ExternalInput")
    wk = nc.dram_tensor("wk", [128, 8, HD], DT, kind="ExternalInput")
    wv = nc.dram_tensor("wv", [128, 8, HD], DT, kind="ExternalInput")
    wo = nc.dram_tensor("wo", [128, 2, E], DT, kind="ExternalInput")
    cosd = nc.dram_tensor("cosd", [128, S], DT, kind="ExternalInput")
    sins = nc.dram_tensor("sins", [128, S], DT, kind="ExternalInput")
    trim = nc.dram_tensor("trim", [128, 128], DT, kind="ExternalInput")
    sel2c = nc.dram_tensor("sel2c", [33, 128], DT, kind="ExternalInput")
    y = nc.dram_tensor("y", [S, E], DT, kind="ExternalOutput")

    with tile.TileContext(nc) as tc:
        with (
            tc.tile_pool(name="persist", bufs=1) as pp,
            tc.tile_pool(name="xchunks", bufs=3) as xp,
            tc.tile_pool(name="ropetmp", bufs=4) as rt,
            tc.tile_pool(name="att_es", bufs=3) as ep,
            tc.tile_pool(name="att_row", bufs=2) as rp,
            tc.tile_pool(name="osb", bufs=4) as op_,
            tc.tile_pool(name="ystg", bufs=2) as yp,
            tc.tile_pool(name="ps_big", bufs=3, space="PSUM") as psB,
            tc.tile_pool(name="ps_ot", bufs=2, space="PSUM") as psO,
        ):
            # ---- persistent tensors ----
            # Qr^T zero-padded per head half: qz[:, hi, blk, s] has rows of
            # head 2*blk+hi live and the other 64 rows zero, so the score
            # contraction runs over the full 128 partitions (PE array dense).
            qz = pp.tile([128, 2, 2, S], DT)
            krt = pp.tile([128, 2, S], DT)   # Kr^T
            vau = pp.tile([128, NTB, HG, 65], DT)  # V + ones col per (tb, h)
            ot = pp.tile([128, 2, S], DT)    # O^T normalized

            # stationary for PE-warming matmuls; zero/one inits run on the
            # otherwise-idle GpSimd so the DVE queue stays clear for real
            # work (vau copies gate the projection PSUM ring).
            hW = pp.tile([128, 128], DT)
            nc.gpsimd.memset(hW[:], 1.0)

            def heat(n=10):
                # full-array 128x128 matmuls to trip the HAM activity window
                # back to K=8/8. Scratch lands in a big-ring PSUM slot whose
                # next real matmul uses start=True and overwrites it.
                htile = psB.tile([128, 128], F32, tag="big", name="heat")
                for _ in range(n):
                    nc.tensor.matmul(htile[:], hW[:], hW[:],
                                     start=True, stop=True)

            # Initial loads fan out over both hardware DMA queues so the
            # first-needed tensors don't wait behind the rest.
            wv_sb = pp.tile([128, 8, HD], DT)
            nc.sync.dma_start(wv_sb[:], wv.ap())
            xc0 = xp.tile([128, 8, 512], DT, tag="xc", name="xc0")
            nc.scalar.dma_start(xc0[:], xT.ap()[:, 0, :, :])
            wq_sb = pp.tile([128, 8, HD], DT)
            nc.sync.dma_start(wq_sb[:], wq.ap())
            wk_sb = pp.tile([128, 8, HD], DT)
            nc.scalar.dma_start(wk_sb[:], wk.ap())
            cos_sb = pp.tile([128, S], DT)
            nc.scalar.dma_start(cos_sb[:], cosd.ap())
            sin_sb = pp.tile([128, S], DT)
            nc.scalar.dma_start(sin_sb[:], sins.ap())
            tri_sb = pp.tile([128, 128], DT)
            nc.sync.dma_start(tri_sb[:], trim.ap())
            sel2_sb = pp.tile([33, 128], DT)
            nc.sync.dma_start(sel2_sb[:], sel2c.ap())
            wo_sb = pp.tile([128, 2, E], DT)
            nc.sync.dma_start(wo_sb[:], wo.ap())

            rows_t = pp.tile([33, 512], DT)
            nc.gpsimd.memset(rows_t[:], 0.0)
            nc.gpsimd.memset(qz[64:128, 0, :, :], 0.0)
            nc.gpsimd.memset(qz[0:64, 1, :, :], 0.0)
            nc.gpsimd.memset(vau[:, :, :, 64:65], 1.0)

            # warm the PE during the initial DMA streams
            heat(n=48)

            def emit_norm(sc, hp, osb, heat_before_bc=0):
                # softmax 1/colsum via ACT exp(-ln); both heads' recip rows
                # land in rows_t, one selector matmul broadcasts them to the
                # full 128-block, then DVE scales the O rows.
                ss = slice(sc * 512, (sc + 1) * 512)
                for hi in range(2):
                    lnr = rp.tile([1, 512], F32, tag="lnr", name=f"lnr{hi}")
                    nc.scalar.activation(lnr[:], osb[hi][64:65, :], Ln, bias=0.0, scale=1.0)
                    with nc.allow_low_precision(reason="recip broadcast row"):
                        nc.scalar.activation(rows_t[32 * hi:32 * hi + 1, :], lnr[:],
                                             Exp, bias=0.0, scale=-1.0)
                if heat_before_bc:
                    heat(n=heat_before_bc)
                bc = psB.tile([128, 512], F32, tag="big", name="bc")
                nc.tensor.matmul(bc[:], sel2_sb[:], rows_t[:], start=True, stop=True)
                for hi in range(2):
                    with nc.allow_low_precision(reason="normalized O rows"):
                        nc.vector.tensor_tensor(ot[hi * 64:(hi + 1) * 64, hp, ss],
                                                osb[hi][0:64, :],
                                                bc[hi * 64:(hi + 1) * 64, :], MUL)

            def emit_wo(sc):
                # W_o partials for chunk sc's 4 s-blocks; both 512-col E
                # halves accumulate in one [128,1024] PSUM pair per s-block.
                for sbl in range(4):
                    sb_i = sc * 4 + sbl
                    tsl = slice(sb_i * 128, (sb_i + 1) * 128)
                    py = psB.tile([128, 1024], F32, tag="big", name="py")
                    for ec in range(2):
                        for blk in range(2):
                            nc.tensor.matmul(
                                py[:, ec * 512:(ec + 1) * 512], ot[:, blk, tsl],
                                wo_sb[:, blk, ec * 512:(ec + 1) * 512],
                                start=(blk == 0), stop=(blk == 1),
                            )
                    ystg = yp.tile([128, E], DT, tag="y")
                    with nc.allow_low_precision(reason="partial sum staging"):
                        nc.vector.tensor_copy(ystg[:], py[:])
                    if sc == 3:
                        heat(n=2)
                    nc.sync.dma_start(y.ap()[tsl, :], ystg[:])

            pend_norm = None  # (sc, osb) for hp=1, normalized next chunk

            for sc in range(NCHUNK):
                ss = slice(sc * 512, (sc + 1) * 512)
                ntb = 4 * sc + 4

                # ---- projection for this chunk: V first (its PSUM->SBUF
                # copies beat the RoPE ops into the DVE queue), then Q, K.
                if sc == 0:
                    xc = xc0
                else:
                    xc = xp.tile([128, 8, 512], DT, tag="xc")
                    nc.sync.dma_start(xc[:], xT.ap()[:, sc, :, :])
                heat(n=2)

                for tbl in range(4):
                    tb = sc * 4 + tbl
                    pv = psB.tile([128, 256], F32, tag="big", name="pv")
                    for e in range(8):
                        nc.tensor.matmul(
                            pv[:], xc[:, e, tbl * 128:(tbl + 1) * 128],
                            wv_sb[:, e, :], start=(e == 0), stop=(e == 7),
                        )
                    with nc.allow_low_precision(reason="rounded matmul input"):
                        nc.vector.tensor_copy(
                            vau[:, tb, :, 0:64],
                            pv[:].rearrange("p (h d) -> p h d", d=64),
                        )

                # Q and K projections with RoPE. The RoPE chain runs per
                # 128-chan block so each half's serial ACT+DVE tail overlaps
                # the other half's matmuls (shorter latency into attention).
                for w_sb, dst in ((wq_sb, None), (wk_sb, krt)):
                    pq = psB.tile([128, 1024], F32, tag="big", name="pq")
                    for mb in range(2):
                        for e in range(8):
                            nc.tensor.matmul(
                                pq[:, mb * 512:(mb + 1) * 512],
                                w_sb[:, e, mb * 128:(mb + 1) * 128],
                                xc[:, e, :], start=(e == 0), stop=(e == 7),
                            )
                        a = rt.tile([128, 512], DT, tag="a")
                        with nc.allow_low_precision(reason="rounded matmul input"):
                            nc.scalar.copy(a[:], pq[:, mb * 512:(mb + 1) * 512])
                        bsh = rt.tile([128, 512], DT, tag="b")
                        nc.vector.stream_shuffle(bsh[:], a[:], SHUF16)
                        t1 = rt.tile([128, 512], DT, tag="t1")
                        t2 = rt.tile([128, 512], DT, tag="t2")
                        with nc.allow_low_precision(reason="rounded matmul input"):
                            nc.vector.tensor_tensor(t1[:], bsh[:], sin_sb[:, ss], MUL)
                            nc.vector.tensor_tensor(t2[:], a[:], cos_sb[:, ss], MUL)
                            if dst is None:  # Q: split into zero-padded halves
                                nc.vector.tensor_tensor(
                                    qz[0:64, 0, mb, ss], t2[0:64, :], t1[0:64, :], ADD)
                                nc.vector.tensor_tensor(
                                    qz[64:128, 1, mb, ss], t2[64:128, :], t1[64:128, :], ADD)
                            else:
                                nc.vector.tensor_tensor(dst[:, mb, ss], t2[:], t1[:], ADD)
                    if w_sb is wq_sb and pend_norm is not None:
                        # normalize previous chunk's hp=1 while K streams
                        emit_norm(pend_norm[0], 1, pend_norm[1])
                        pend_norm = None

                # ---- attention for this chunk, one head-pair at a time ----
                osb_h = [None, None]
                for hp in range(2):
                    otp = [psO.tile([65, 512], F32, tag="ot", name=f"otp{hi}")
                           for hi in range(2)]
                    blk = hp
                    for tb in range(ntb):
                        m = tb - 4 * sc
                        lo = 128 * max(m, 0)  # diag: skip cols left of block
                        if sc == 3 and tb % 5 == 1:
                            heat(n=5)
                        if hp == 1 and tb == 2:
                            # deferred normalize of this chunk's hp=0
                            emit_norm(sc, 0, osb_h[0])
                        pss = psB.tile([128, 1024], F32, tag="big", name="pss")
                        ps3 = pss[:].rearrange("p (h s) -> p h s", h=2)
                        for hi in range(2):
                            nc.tensor.matmul(
                                ps3[:, hi, lo:512],
                                krt[:, blk, tb * 128:(tb + 1) * 128],
                                qz[:, hi, blk, sc * 512 + lo:(sc + 1) * 512],
                                start=True, stop=True,
                            )
                        es = ep.tile([128, 1024], DT, tag="es", name="es")
                        es3 = es[:].rearrange("p (h s) -> p h s", h=2)
                        with nc.allow_low_precision(reason="rounded matmul input"):
                            nc.scalar.activation(es3[:, :, lo:512], ps3[:, :, lo:512],
                                                 Exp, bias=0.0, scale=SCALE)
                        if m >= 0:  # mask the diagonal 128-col triangle
                            trib = tri_sb[:].rearrange("p (o s) -> p o s", o=1).to_broadcast((128, 2, 128))
                            with nc.allow_low_precision(reason="rounded matmul input"):
                                nc.vector.tensor_tensor(
                                    es3[:, :, lo:lo + 128], es3[:, :, lo:lo + 128],
                                    trib, MUL)
                        for hi in range(2):
                            nc.tensor.matmul(
                                otp[hi][:, lo:512], vau[:, tb, 2 * hp + hi, :],
                                es[:, hi * 512 + lo:(hi + 1) * 512],
                                start=(tb == 0), stop=(tb == ntb - 1),
                                skip_group_check=True,
                            )
                    # drain O accumulators to SBUF promptly so the two PSUM
                    # banks recycle for the next head-pair.
                    osb = [op_.tile([65, 512], DT, tag="osb", name=f"osb{hi}")
                           for hi in range(2)]
                    for hi in range(2):
                        with nc.allow_low_precision(reason="pre-normalize O"):
                            nc.scalar.copy(osb[hi][:], otp[hi][:])
                    osb_h[hp] = osb

                    # W_o of the previous chunk after hp0: the DVE queue has
                    # drained the projection RoPE by now, so its normalize
                    # finished; wo also gives ACT slack to run ahead on exps.
                    if hp == 0 and sc > 0:
                        emit_wo(sc - 1)

                pend_norm = (sc, osb_h[1])
                if sc == 3:
                    # final chunk: normalize hp=1 and emit its W_o inside the
                    # loop body (past the loop there is a block boundary whose
                    # cross-engine drain would serialize the tail).
                    emit_norm(3, 1, pend_norm[1], heat_before_bc=24)
                    emit_wo(3)

    _legalize_waits(nc)
    return nc


def _legalize_waits(nc, max_waits=1):
    """Split >max_waits sync waits onto preceding same-engine NoOps
    (several instruction encodings only have one sync-wait slot)."""
    for fn in nc.m.functions:
        for bb in fn.blocks:
            new_insts = []
            for inst in bb.instructions:
                si = inst.sync_info
                waits = list(si.on_wait) if si is not None and si.on_wait else []
                if len(waits) > max_waits:
                    carry, keep = waits[:-max_waits], waits[-max_waits:]
                    for i, w in enumerate(carry):
                        new_insts.append(mybir.InstNoOp(
                            name=f"{inst.name}_wsplit{i}",
                            engine=inst.engine,
                            bass_nofuse=True,
                            sync_info=mybir.SyncInfo(on_wait=[w], on_update=[]),
                        ))
                    si.on_wait = keep
                new_insts.append(inst)
            bb.instructions[:] = new_insts


def _host_constants():
    # RoPE channel permutation: row r (within a head, 0..63) holds source
    # channel d = 2*i + odd with i = 16*(r//32) + r%16, odd = (r%32)//16.
    r = np.arange(64)
    i_ = 16 * (r // 32) + (r % 16)
    odd = (r % 32) // 16
    dsrc = 2 * i_ + odd  # source channel per permuted row

    inv_freq = ROPE_BASE ** (-(i_.astype(np.float64)) * 2.0 / Dh)
    ang = np.arange(S, dtype=np.float64)[None, :] * inv_freq[:, None]  # [64, S]
    cos64 = np.cos(ang)
    sin64 = np.sin(ang) * np.where(odd == 0, -1.0, 1.0)[:, None]
    cosd = np.tile(cos64, (2, 1)).astype(DT_NP)
    sins = np.tile(sin64, (2, 1)).astype(DT_NP)

    t = np.arange(128)[:, None]
    s = np.arange(128)[None, :]
    trim = (t <= s).astype(DT_NP)

    sel2 = np.zeros((33, 128), DT_NP)
    sel2[0, 0:64] = 1
    sel2[32, 64:128] = 1
    return dsrc, cosd, sins, trim, sel2


def _wlay(w):  # [E, HD] -> [p, ko, m] contiguous
    return np.ascontiguousarray(w.reshape(8, 128, HD).transpose(1, 0, 2)).astype(DT_NP)


def _wolay(w):  # [HD, E] -> [p, ko, e] contiguous
    return np.ascontiguousarray(w.reshape(2, 128, E).transpose(1, 0, 2)).astype(DT_NP)


_CACHE = {}


def _run(inputs, trace=False):
    if "nc" not in _CACHE:
        _CACHE["nc"] = _build_program()
        _CACHE["consts"] = _host_constants()
    nc = _CACHE["nc"]
    dsrc, cosd, sins, trim, sel2 = _CACHE["consts"]

    x = np.ascontiguousarray(np.asarray(inputs["x"]), dtype=np.float32)
    W_q = np.asarray(inputs["W_q"], dtype=np.float32)
    W_k = np.asarray(inputs["W_k"], dtype=np.float32)
    W_v = np.asarray(inputs["W_v"], dtype=np.float32)
    W_o = np.asarray(inputs["W_o"], dtype=np.float32)

    # [p, sc, eo, s] so each chunk DMA is 8KB contiguous per partition
    xT = [np.ascontiguousarray(
        x[b].reshape(NCHUNK, 512, 8, 128).transpose(3, 0, 2, 1)).astype(DT_NP)
        for b in range(B)]

    in_maps = []
    for c in range(8):
        b, g = divmod(c, 4)
        heads = np.arange(4 * g, 4 * g + 4)
        rows_qk = (heads[:, None] * 64 + dsrc[None, :]).reshape(-1)   # permuted
        rows_v = (heads[:, None] * 64 + np.arange(64)[None, :]).reshape(-1)
        in_maps.append({
            "xT": xT[b],
            "wq": _wlay(W_q[rows_qk].T),
            "wk": _wlay(W_k[rows_qk].T),
            "wv": _wlay(W_v[rows_v].T),
            "wo": _wolay(W_o[:, rows_v].T),
            "cosd": cosd, "sins": sins, "trim": trim, "sel2c": sel2,
        })

    res = bass_utils.run_bass_kernel_spmd(
        nc, in_maps, core_ids=list(range(8)), trace=trace,
    )
    out = np.zeros((B, S, E), np.float32)
    for c in range(8):
        out[c // 4] += res.results[c]["y"].astype(np.float32)
    return out, res


def kernel(**inputs):
    out, _ = _run(inputs, trace=False)
    return out


# revision 26
# speedup vs baseline: 1.1454x; 1.1454x over previous
"""Multi-head causal attention with RoPE on 8 TRN2 NeuronCores.

Sharding: batch (2) x head-groups (4 of 4 heads) -> 8 cores.
Per core, processed per 512-row s-chunk with everything interleaved to keep
the PE array dense (HAM stays at K=8/8): QKV projection for the chunk,
RoPE (stream_shuffle + sign-folded cos/sin), transposed scores
S^T = Kr @ Qr^T with both heads of a pair in one [128,1024] PSUM tile (the
heads share the Kr-block stationary; one fused scale+exp covers the pair),
causal block-skip plus column-subrange matmuls on the diagonal blocks, PV
matmul with a ones-column on V accumulating the softmax denominator,
ACT-side reciprocal (exp(-ln)), ones-matmul broadcast. O accumulators are
copied out of PSUM right after each head-pair (ACT) so two PSUM banks
suffice; the normalize chain runs deferred off the Tensor critical path,
and W_o runs a half-chunk behind. Host sums the 4 per-batch partials.
bf16 throughout; f32 PSUM accumulation.
"""
import os
import sys

sys.path.insert(0, "/opt/trn_rl_repo")

import ml_dtypes
import numpy as np

import concourse.bass as bass
import concourse.mybir as mybir
import concourse.tile as tile
from concourse import bass_utils

F32 = mybir.dt.float32
F32R = mybir.dt.float32r
BF16 = mybir.dt.bfloat16

DT_NAME = os.environ.get("ATTN_DT", "bf16")
DT = {"f32r": F32R, "bf16": BF16}[DT_NAME]
DT_NP = {"f32r": np.float32, "bf16": ml_dtypes.bfloat16}[DT_NAME]

B, S, E, H, Dh = 2, 2048, 1024, 16, 64
HG = 4            # heads per core
HD = HG * Dh      # 256 output channels per core
SCALE = float(1.0 / np.sqrt(np.float32(1024.0)))
ROPE_BASE = 10000.0
NCHUNK = S // 512     # 4 s-chunks of 512
NTB = S // 128        # 16 t-blocks of 128
SHUF16 = list(range(16, 32)) + list(range(0, 16))

Exp = mybir.ActivationFunctionType.Exp
Ln = mybir.ActivationFunctionType.Ln
MUL = mybir.AluOpType.mult
ADD = mybir.AluOpType.add


def _build_program():
    nc = bass.Bass("TRN2", target_bir_lowering=False, debug=False)

    xT = nc.dram_tensor("xT", [128, NCHUNK, 8, 512], DT, kind="ExternalInput")
    wq = nc.dram_tensor("wq", [128, 8, HD], DT, kind="ExternalInput")
    wk = nc.dram_tensor("wk", [128, 8, HD], DT, kind="ExternalInput")
    wv = nc.dram_tensor("wv", [128, 8, HD], DT, kind="ExternalInput")
    wo = nc.dram_tensor("wo", [128, 2, E], DT, kind="ExternalInput")
    cosd = nc.dram_tensor("cosd", [128, S], DT, kind="ExternalInput")
    sins = nc.dram_tensor("sins", [128, S], DT, kind="ExternalInput")
    trim = nc.dram_tensor("trim", [128, 128], DT, kind="ExternalInput")
    sel2c = nc.dram_tensor("sel2c", [33, 128], DT, kind="ExternalInput")
    y = nc.dram_tensor("y", [S, E], DT, kind="ExternalOutput")

    with tile.TileContext(nc) as tc:
        with (
            tc.tile_pool(name="persist", bufs=1) as pp,
            tc.tile_pool(name="xchunks", bufs=3) as xp,
            tc.tile_pool(name="ropetmp", bufs=2) as rt,
            tc.tile_pool(name="att_es", bufs=3) as ep,
            tc.tile_pool(name="att_row", bufs=2) as rp,
            tc.tile_pool(name="osb", bufs=4) as op_,
            tc.tile_pool(name="ystg", bufs=2) as yp,
            tc.tile_pool(name="ps_big", bufs=3, space="PSUM") as psB,
            tc.tile_pool(name="ps_ot", bufs=2, space="PSUM") as psO,
        ):
            # ---- persistent tensors ----
            # Qr^T zero-padded per head half: qz[:, hi, blk, s] has rows of
            # head 2*blk+hi live and the other 64 rows zero, so the score
            # contraction runs over the full 128 partitions (PE array dense).
            qz = pp.tile([128, 2, 2, S], DT)
            krt = pp.tile([128, 2, S], DT)   # Kr^T
            vau = pp.tile([128, NTB, HG, 65], DT)  # V + ones col per (tb, h)
            ot = pp.tile([128, 2, S], DT)    # O^T normalized

            # stationary for PE-warming matmuls; zero/one inits run on the
            # otherwise-idle GpSimd so the DVE queue stays clear for real
            # work (vau copies gate the projection PSUM ring).
            hW = pp.tile([128, 128], DT)
            nc.gpsimd.memset(hW[:], 1.0)

            def heat(n=10):
                # full-array 128x128 matmuls to trip the HAM activity window
                # back to K=8/8. Scratch lands in a big-ring PSUM slot whose
                # next real matmul uses start=True and overwrites it.
                htile = psB.tile([128, 128], F32, tag="big", name="heat")
                for _ in range(n):
                    nc.tensor.matmul(htile[:], hW[:], hW[:],
                                     start=True, stop=True)

            # Initial loads fan out over both hardware DMA queues so the
            # first-needed tensors don't wait behind the rest.
            wv_sb = pp.tile([128, 8, HD], DT)
            nc.sync.dma_start(wv_sb[:], wv.ap())
            xc0 = xp.tile([128, 8, 512], DT, tag="xc", name="xc0")
            nc.scalar.dma_start(xc0[:], xT.ap()[:, 0, :, :])
            wq_sb = pp.tile([128, 8, HD], DT)
            nc.sync.dma_start(wq_sb[:], wq.ap())
            wk_sb = pp.tile([128, 8, HD], DT)
            nc.scalar.dma_start(wk_sb[:], wk.ap())
            cos_sb = pp.tile([128, S], DT)
            nc.scalar.dma_start(cos_sb[:], cosd.ap())
            sin_sb = pp.tile([128, S], DT)
            nc.scalar.dma_start(sin_sb[:], sins.ap())
            tri_sb = pp.tile([128, 128], DT)
            nc.sync.dma_start(tri_sb[:], trim.ap())
            sel2_sb = pp.tile([33, 128], DT)
            nc.sync.dma_start(sel2_sb[:], sel2c.ap())
            wo_sb = pp.tile([128, 2, E], DT)
            nc.sync.dma_start(wo_sb[:], wo.ap())

            rows_t = pp.tile([33, 512], DT)
            nc.gpsimd.memset(rows_t[:], 0.0)
            nc.gpsimd.memset(qz[64:128, 0, :, :], 0.0)
            nc.gpsimd.memset(qz[0:64, 1, :, :], 0.0)
            nc.gpsimd.memset(vau[:, :, :, 64:65], 1.0)

            # warm the PE during the initial DMA streams
            heat(n=48)

            def emit_norm(sc, hp, osb, heat_before_bc=0):
                # softmax 1/colsum via ACT exp(-ln); both heads' recip rows
                # land in rows_t, one selector matmul broadcasts them to the
                # full 128-block, then DVE scales the O rows.
                ss = slice(sc * 512, (sc + 1) * 512)
                for hi in range(2):
                    lnr = rp.tile([1, 512], F32, tag="lnr", name=f"lnr{hi}")
                    nc.scalar.activation(lnr[:], osb[hi][64:65, :], Ln, bias=0.0, scale=1.0)
                    with nc.allow_low_precision(reason="recip broadcast row"):
                        nc.scalar.activation(rows_t[32 * hi:32 * hi + 1, :], lnr[:],
                                             Exp, bias=0.0, scale=-1.0)
                if heat_before_bc:
                    heat(n=heat_before_bc)
                bc = psB.tile([128, 512], F32, tag="big", name="bc")
                nc.tensor.matmul(bc[:], sel2_sb[:], rows_t[:], start=True, stop=True)
                for hi in range(2):
                    with nc.allow_low_precision(reason="normalized O rows"):
                        nc.vector.tensor_tensor(ot[hi * 64:(hi + 1) * 64, hp, ss],
                                                osb[hi][0:64, :],
                                                bc[hi * 64:(hi + 1) * 64, :], MUL)

            def emit_wo(sc):
                # W_o partials for chunk sc's 4 s-blocks; both 512-col E
                # halves accumulate in one [128,1024] PSUM pair per s-block.
                for sbl in range(4):
                    sb_i = sc * 4 + sbl
                    tsl = slice(sb_i * 128, (sb_i + 1) * 128)
                    py = psB.tile([128, 1024], F32, tag="big", name="py")
                    for ec in range(2):
                        for blk in range(2):
                            nc.tensor.matmul(
                                py[:, ec * 512:(ec + 1) * 512], ot[:, blk, tsl],
                                wo_sb[:, blk, ec * 512:(ec + 1) * 512],
                                start=(blk == 0), stop=(blk == 1),
                            )
                    ystg = yp.tile([128, E], DT, tag="y")
                    with nc.allow_low_precision(reason="partial sum staging"):
                        nc.vector.tensor_copy(ystg[:], py[:])
                    if sc == 3:
                        heat(n=2)
                    nc.sync.dma_start(y.ap()[tsl, :], ystg[:])

            pend_norm = None  # (sc, osb) for hp=1, normalized next chunk

            for sc in range(NCHUNK):
                ss = slice(sc * 512, (sc + 1) * 512)
                ntb = 4 * sc + 4

                # ---- projection for this chunk: V first (its PSUM->SBUF
                # copies beat the RoPE ops into the DVE queue), then Q, K.
                if sc == 0:
                    xc = xc0
                else:
                    xc = xp.tile([128, 8, 512], DT, tag="xc")
                    nc.sync.dma_start(xc[:], xT.ap()[:, sc, :, :])
                heat(n=2)

                for tbl in range(4):
                    tb = sc * 4 + tbl
                    pv = psB.tile([128, 256], F32, tag="big", name="pv")
                    for e in range(8):
                        nc.tensor.matmul(
                            pv[:], xc[:, e, tbl * 128:(tbl + 1) * 128],
                            wv_sb[:, e, :], start=(e == 0), stop=(e == 7),
                        )
                    with nc.allow_low_precision(reason="rounded matmul input"):
                        nc.vector.tensor_copy(
                            vau[:, tb, :, 0:64],
                            pv[:].rearrange("p (h d) -> p h d", d=64),
                        )

                # Q and K projections with RoPE, both 128-chan blocks of a
                # projection in one [128,1024] PSUM pair.
                for w_sb, dst in ((wq_sb, None), (wk_sb, krt)):
                    pq = psB.tile([128, 1024], F32, tag="big", name="pq")
                    for mb in range(2):
                        for e in range(8):
                            nc.tensor.matmul(
                                pq[:, mb * 512:(mb + 1) * 512],
                                w_sb[:, e, mb * 128:(mb + 1) * 128],
                                xc[:, e, :], start=(e == 0), stop=(e == 7),
                            )
                    a = rt.tile([128, 1024], DT, tag="a")
                    with nc.allow_low_precision(reason="rounded matmul input"):
                        nc.scalar.copy(a[:], pq[:])
                    bsh = rt.tile([128, 1024], DT, tag="b")
                    nc.vector.stream_shuffle(bsh[:], a[:], SHUF16)
                    t1 = rt.tile([128, 1024], DT, tag="t1")
                    t2 = rt.tile([128, 1024], DT, tag="t2")
                    cosb = cos_sb[:, ss].rearrange("p (o s) -> p o s", o=1).to_broadcast((128, 2, 512))
                    sinb = sin_sb[:, ss].rearrange("p (o s) -> p o s", o=1).to_broadcast((128, 2, 512))
                    a3 = a[:].rearrange("p (m s) -> p m s", m=2)
                    b3 = bsh[:].rearrange("p (m s) -> p m s", m=2)
                    t13 = t1[:].rearrange("p (m s) -> p m s", m=2)
                    t23 = t2[:].rearrange("p (m s) -> p m s", m=2)
                    with nc.allow_low_precision(reason="rounded matmul input"):
                        nc.vector.tensor_tensor(t13, b3, sinb, MUL)
                        nc.vector.tensor_tensor(t23, a3, cosb, MUL)
                        if dst is None:  # Q: split into zero-padded halves
                            nc.vector.tensor_tensor(
                                qz[0:64, 0, :, ss],
                                t2[0:64, :].rearrange("p (m s) -> p m s", m=2),
                                t1[0:64, :].rearrange("p (m s) -> p m s", m=2), ADD)
                            nc.vector.tensor_tensor(
                                qz[64:128, 1, :, ss],
                                t2[64:128, :].rearrange("p (m s) -> p m s", m=2),
                                t1[64:128, :].rearrange("p (m s) -> p m s", m=2), ADD)
                        else:
                            nc.vector.tensor_tensor(dst[:, :, ss], t23, t13, ADD)
                    if w_sb is wq_sb and pend_norm is not None:
                        # normalize previous chunk's hp=1 while K streams
                        emit_norm(pend_norm[0], 1, pend_norm[1])
                        pend_norm = None

                # ---- attention for this chunk, one head-pair at a time ----
                osb_h = [None, None]
                for hp in range(2):
                    otp = [psO.tile([65, 512], F32, tag="ot", name=f"otp{hi}")
                           for hi in range(2)]
                    blk = hp
                    for tb in range(ntb):
                        m = tb - 4 * sc
                        lo = 128 * max(m, 0)  # diag: skip cols left of block
                        if sc == 3 and tb % 5 == 1:
                            heat(n=5)
                        if hp == 1 and tb == 2:
                            # deferred normalize of this chunk's hp=0
                            emit_norm(sc, 0, osb_h[0])
                        pss = psB.tile([128, 1024], F32, tag="big", name="pss")
                        ps3 = pss[:].rearrange("p (h s) -> p h s", h=2)
                        for hi in range(2):
                            nc.tensor.matmul(
                                ps3[:, hi, lo:512],
                                krt[:, blk, tb * 128:(tb + 1) * 128],
                                qz[:, hi, blk, sc * 512 + lo:(sc + 1) * 512],
                                start=True, stop=True,
                            )
                        es = ep.tile([128, 1024], DT, tag="es", name="es")
                        es3 = es[:].rearrange("p (h s) -> p h s", h=2)
                        with nc.allow_low_precision(reason="rounded matmul input"):
                            nc.scalar.activation(es3[:, :, lo:512], ps3[:, :, lo:512],
                                                 Exp, bias=0.0, scale=SCALE)
                        if m >= 0:  # mask the diagonal 128-col triangle
                            trib = tri_sb[:].rearrange("p (o s) -> p o s", o=1).to_broadcast((128, 2, 128))
                            with nc.allow_low_precision(reason="rounded matmul input"):
                                nc.vector.tensor_tensor(
                                    es3[:, :, lo:lo + 128], es3[:, :, lo:lo + 128],
                                    trib, MUL)
                        for hi in range(2):
                            nc.tensor.matmul(
                                otp[hi][:, lo:512], vau[:, tb, 2 * hp + hi, :],
                                es[:, hi * 512 + lo:(hi + 1) * 512],
                                start=(tb == 0), stop=(tb == ntb - 1),
                                skip_group_check=True,
                            )
                    # drain O accumulators to SBUF promptly so the two PSUM
                    # banks recycle for the next head-pair.
                    osb = [op_.tile([65, 512], DT, tag="osb", name=f"osb{hi}")
                           for hi in range(2)]
                    for hi in range(2):
                        with nc.allow_low_precision(reason="pre-normalize O"):
                            nc.scalar.copy(osb[hi][:], otp[hi][:])
                    osb_h[hp] = osb

                    # W_o of the previous chunk after hp0: the DVE queue has
                    # drained the projection RoPE by now, so its normalize
                    # finished; wo also gives ACT slack to run ahead on exps.
                    if hp == 0 and sc > 0:
                        emit_wo(sc - 1)

                pend_norm = (sc, osb_h[1])
                if sc == 3:
                    emit_norm(3, 1, pend_norm[1], heat_before_bc=24)
                    emit_wo(3)

    _legalize_waits(nc)
    return nc


def _legalize_waits(nc, max_waits=1):
    """Split >max_waits sync waits onto preceding same-engine NoOps
    (several instruction encodings only have one sync-wait slot)."""
    for fn in nc.m.functions:
        for bb in fn.blocks:
            new_insts = []
            for inst in bb.instructions:
                si = inst.sync_info
                waits = list(si.on_wait) if si is not None and si.on_wait else []
                if len(waits) > max_waits:
                    carry, keep = waits[:-max_waits], waits[-max_waits:]
                    for i, w in enumerate(carry):
                        new_insts.append(mybir.InstNoOp(
                            name=f"{inst.name}_wsplit{i}",
                            engine=inst.engine,
                            bass_nofuse=True,
                            sync_info=mybir.SyncInfo(on_wait=[w], on_update=[]),
                        ))
                    si.on_wait = keep
                new_insts.append(inst)
            bb.instructions[:] = new_insts


def _host_constants():
    # RoPE channel permutation: row r (within a head, 0..63) holds source
    # channel d = 2*i + odd with i = 16*(r//32) + r%16, odd = (r%32)//16.
    r = np.arange(64)
    i_ = 16 * (r // 32) + (r % 16)
    odd = (r % 32) // 16
    dsrc = 2 * i_ + odd  # source channel per permuted row

    inv_freq = ROPE_BASE ** (-(i_.astype(np.float64)) * 2.0 / Dh)
    ang = np.arange(S, dtype=np.float64)[None, :] * inv_freq[:, None]  # [64, S]
    cos64 = np.cos(ang)
    sin64 = np.sin(ang) * np.where(odd == 0, -1.0, 1.0)[:, None]
    cosd = np.tile(cos64, (2, 1)).astype(DT_NP)
    sins = np.tile(sin64, (2, 1)).astype(DT_NP)

    t = np.arange(128)[:, None]
    s = np.arange(128)[None, :]
    trim = (t <= s).astype(DT_NP)

    sel2 = np.zeros((33, 128), DT_NP)
    sel2[0, 0:64] = 1
    sel2[32, 64:128] = 1
    return dsrc, cosd, sins, trim, sel2


def _wlay(w):  # [E, HD] -> [p, ko, m] contiguous
    return np.ascontiguousarray(w.reshape(8, 128, HD).transpose(1, 0, 2)).astype(DT_NP)


def _wolay(w):  # [HD, E] -> [p, ko, e] contiguous
    return np.ascontiguousarray(w.reshape(2, 128, E).transpose(1, 0, 2)).astype(DT_NP)


_CACHE = {}


def _run(inputs, trace=False):
    if "nc" not in _CACHE:
        _CACHE["nc"] = _build_program()
        _CACHE["consts"] = _host_constants()
    nc = _CACHE["nc"]
    dsrc, cosd, sins, trim, sel2 = _CACHE["consts"]

    x = np.ascontiguousarray(np.asarray(inputs["x"]), dtype=np.float32)
    W_q = np.asarray(inputs["W_q"], dtype=np.float32)
    W_k = np.asarray(inputs["W_k"], dtype=np.float32)
    W_v = np.asarray(inputs["W_v"], dtype=np.float32)
    W_o = np.asarray(inputs["W_o"], dtype=np.float32)

    # [p, sc, eo, s] so each chunk DMA is 8KB contiguous per partition
    xT = [np.ascontiguousarray(
        x[b].reshape(NCHUNK, 512, 8, 128).transpose(3, 0, 2, 1)).astype(DT_NP)
        for b in range(B)]

    in_maps = []
    for c in range(8):
        b, g = divmod(c, 4)
        heads = np.arange(4 * g, 4 * g + 4)
        rows_qk = (heads[:, None] * 64 + dsrc[None, :]).reshape(-1)   # permuted
        rows_v = (heads[:, None] * 64 + np.arange(64)[None, :]).reshape(-1)
        in_maps.append({
            "xT": xT[b],
            "wq": _wlay(W_q[rows_qk].T),
            "wk": _wlay(W_k[rows_qk].T),
            "wv": _wlay(W_v[rows_v].T),
            "wo": _wolay(W_o[:, rows_v].T),
            "cosd": cosd, "sins": sins, "trim": trim, "sel2c": sel2,
        })

    res = bass_utils.run_bass_kernel_spmd(
        nc, in_maps, core_ids=list(range(8)), trace=trace,
    )
    out = np.zeros((B, S, E), np.float32)
    for c in range(8):
        out[c // 4] += res.results[c]["y"].astype(np.float32)
    return out, res


def kernel(**inputs):
    out, _ = _run(inputs, trace=False)
    return out
